# revision 1
# baseline (speedup 1.0000x reference)
"""Trainium2 Bass kernel for the CAA (channel-affinity attention) module.

Reference computation per sample b (C=1024 channels, N=256 positions):
    x_hat = x^T                              (N, C)
    q = relu(BN1(Wq @ x_hat))                (64, C)
    k = relu(BN2(Wk @ x_hat))                (64, C)
    sim[c, d] = sum_o k[o, c] * q[o, d]      (C, C)
    aff = softmax(rowmax(sim) - sim, axis d) == softmax(-sim, axis d)
    v = relu(BN3(Wv @ x))                    (C, N)
    out = alpha * (aff @ v) + x              (C, N)

Device-side strategy (pure data parallel, 4 samples per core x 8 cores):
  * BN folded into weights/bias on the host; |alpha| folded into Wv/t3 so
    the softmax denominator comes out as plain Z (ones column appended to v).
  * sim is computed TRANSPOSED (d on partitions, c on free) so the exp(-sim)
    tiles feed the aff @ v contraction directly as matmul lhsT -- no on-chip
    transposes.
  * all matmul operands are bf16 (fp32 PSUM accumulation); the residual add
    reads a full-fp32 copy of x, so the output carries x at full precision
    plus a bf16-accurate alpha*attention term.
  * per-sample phases are software-pipelined in emission order
    (qk -> [U of prev] -> sim -> v) to keep the PE dense.
"""

import os
import sys

import numpy as np

_REPO = "/opt/trn_rl_repo"
if _REPO not in sys.path:
    sys.path.insert(0, _REPO)

import ml_dtypes  # noqa: E402

import concourse.bacc as bacc  # noqa: E402
import concourse.tile as tile  # noqa: E402
from concourse import mybir  # noqa: E402
from concourse.bass_utils import run_bass_kernel_spmd  # noqa: E402
from concourse.tile import add_dep_helper  # noqa: E402

F32 = mybir.dt.float32
BF16 = mybir.dt.bfloat16
AF = mybir.ActivationFunctionType
ALU = mybir.AluOpType
BFNP = ml_dtypes.bfloat16

B, C, N = 32, 1024, 256
DQ = 64
NCORES = 8
BS = B // NCORES  # samples per core
CCH = C // 128    # chunks of the channel dim
KCH = N // 128    # chunks of the position dim (qk contraction)
EPS = 1e-5

LAST_RESULTS = None  # BassKernelResults of the most recent run
_NC_CACHE = {}


def _build(bs: int = BS):
    nc = bacc.Bacc("TRN2", target_bir_lowering=False, debug=False)

    x_d = nc.dram_tensor("x_in", (bs, 128, CCH, N), F32, kind="ExternalInput")
    xb_d = nc.dram_tensor("xb_in", (bs, 128, CCH, N), BF16, kind="ExternalInput")
    xt_d = nc.dram_tensor("xt_in", (bs, 128, KCH, C), BF16, kind="ExternalInput")
    wqkt_d = nc.dram_tensor("wqkt", (128, KCH, 128), BF16, kind="ExternalInput")
    tqk_d = nc.dram_tensor("tqk", (128, 1), F32, kind="ExternalInput")
    wvt_d = nc.dram_tensor("wvt", (128, CCH, C), BF16, kind="ExternalInput")
    t3_d = nc.dram_tensor("t3", (128, CCH), F32, kind="ExternalInput")
    vcol_d = nc.dram_tensor("vcol", (128, CCH, 2), BF16, kind="ExternalInput")
    out_d = nc.dram_tensor("y_out", (bs, 128, CCH, N), F32, kind="ExternalOutput")

    with tile.TileContext(nc) as tc:
        with (
            tc.tile_pool(name="consts", bufs=1) as consts,
            tc.tile_pool(name="xp", bufs=3) as xp,
            tc.tile_pool(name="xbp", bufs=2) as xbp,
            tc.tile_pool(name="xtp", bufs=2) as xtp,
            tc.tile_pool(name="qkp", bufs=2) as qkp,
            tc.tile_pool(name="k0p", bufs=2) as k0p,
            tc.tile_pool(name="etp", bufs=18) as etp,
            tc.tile_pool(name="vp", bufs=2) as vp,
            tc.tile_pool(name="outp", bufs=2) as outp,
            tc.tile_pool(name="smallp", bufs=8) as smallp,
            tc.tile_pool(name="psbig", bufs=2, space="PSUM") as psbig,
            tc.tile_pool(name="psbank", bufs=4, space="PSUM") as psbank,
        ):
            # weights via SWDGE (gpsimd) so they don't block the sync queue
            wqkt = consts.tile([128, KCH, 128], BF16, tag="wqkt")
            nc.gpsimd.dma_start(out=wqkt, in_=wqkt_d[:])
            tqk = consts.tile([128, 1], F32, tag="tqk")
            nc.gpsimd.dma_start(out=tqk, in_=tqk_d[:])
            t3 = consts.tile([128, CCH], F32, tag="t3")
            nc.gpsimd.dma_start(out=t3, in_=t3_d[:])
            vcol = consts.tile([128, CCH, 2], BF16, tag="vcol")
            nc.gpsimd.dma_start(out=vcol, in_=vcol_d[:])
            zero = consts.tile([128, 1], F32, tag="zero")
            nc.vector.memset(zero, 0.0)
            wvt = consts.tile([128, CCH, C], BF16, tag="wvt")
            for h in range(0, CCH, 2):
                nc.gpsimd.dma_start(out=wvt[:, h:h + 2, :], in_=wvt_d[:, h:h + 2, :])
            # touch the activation table early so the lazy ACT_TABLE_LOAD
            # doesn't delay the first (critical-path) relu
            warm = consts.tile([128, 1], F32, tag="warm")
            nc.scalar.activation(out=warm, in_=zero, func=AF.Exp,
                                 bias=zero[:, 0:1], scale=1.0)

            last_sim_mm = [None]
            x_sb = [None] * bs
            xb_sb = [None] * bs
            xt_sb = [None] * bs
            qk_sb = [None] * bs
            k0 = [None] * bs
            v_sb = [None] * bs
            et = [None] * bs

            def load_xt(b):
                # two half-loads: qk's kc=0 matmuls only wait on the first
                xt_sb[b] = xtp.tile([128, KCH, C], BF16, tag="xt", name=f"xt_sb{b}")
                for kc in range(KCH):
                    nc.sync.dma_start(out=xt_sb[b][:, kc, :], in_=xt_d[b, :, kc, :])

            def load_xb(b):
                xb_sb[b] = xbp.tile([128, CCH, N], BF16, tag="xb",
                                    name=f"xb_sb{b}")
                nc.sync.dma_start(out=xb_sb[b], in_=xb_d[b])

            def load_x(b):
                x_sb[b] = xp.tile([128, CCH, N], F32, tag="x", name=f"x_sb{b}")
                nc.sync.dma_start(out=x_sb[b], in_=x_d[b])

            def qk_phase(b):
                # q/k projection: psum rows 0:64 = q, 64:128 = k
                qk_ps = psbig.tile([128, C], F32, tag="psbig")
                for cb in range(C // 512):
                    for kc in range(KCH):
                        nc.tensor.matmul(
                            qk_ps[:, cb * 512:(cb + 1) * 512],
                            wqkt[:, kc, :],
                            xt_sb[b][:, kc, cb * 512:(cb + 1) * 512],
                            start=(kc == 0),
                            stop=(kc == KCH - 1),
                        )
                qk_sb[b] = qkp.tile([128, C], BF16, tag="qk", name=f"qk_sb{b}")
                nc.scalar.activation(
                    out=qk_sb[b], in_=qk_ps, func=AF.Relu,
                    bias=tqk[:, 0:1], scale=1.0,
                )
                # k must sit at partition base 0 to act as matmul rhs
                # issued from the ACT ring: it naturally orders after the
                # relu above and cannot stall the sync ring's bulk loads
                k0[b] = k0p.tile([64, C], BF16, tag="k0", name=f"k0_{b}")
                nc.gpsimd.dma_start(out=k0[b], in_=qk_sb[b][64:128, :])

            def sim_phase(b):
                # transposed sim + exp: et[d][dd, c] = exp(-sim[c, d])
                et[b] = []
                for d in range(CCH):
                    s_ps = psbig.tile([128, C], F32, tag="psbig")
                    for cb in range(C // 512):
                        mm = nc.tensor.matmul(
                            s_ps[:, cb * 512:(cb + 1) * 512],
                            qk_sb[b][0:64, d * 128:(d + 1) * 128],
                            k0[b][:, cb * 512:(cb + 1) * 512],
                            start=True,
                            stop=True,
                        )
                        last_sim_mm[0] = mm.ins
                    e_sb = etp.tile([128, C], BF16, tag="et")
                    nc.scalar.activation(
                        out=e_sb, in_=s_ps, func=AF.Exp,
                        bias=zero[:, 0:1], scale=-1.0,
                    )
                    et[b].append(e_sb)

            def v_phase(b):
                # v = |alpha| * relu(Wv' @ x + t3)  (folded into wvt/t3),
                # plus two sign(alpha) columns (Z accumulator; even width
                # keeps matmul free sizes even)
                v_sb[b] = vp.tile([128, CCH, N + 2], BF16, tag="v",
                                  name=f"v_sb{b}")
                nc.gpsimd.dma_start(out=v_sb[b][:, :, N:N + 2], in_=vcol[:])
                for m in range(CCH):
                    v_ps = psbank.tile([128, N], F32, tag="psbank")
                    for kc in range(CCH):
                        mm = nc.tensor.matmul(
                            v_ps,
                            wvt[:, kc, m * 128:(m + 1) * 128],
                            xb_sb[b][:, kc, :],
                            start=(kc == 0),
                            stop=(kc == CCH - 1),
                        )

                    nc.vector.tensor_scalar(
                        out=v_sb[b][:, m, 0:N],
                        in0=v_ps,
                        scalar1=t3[:, m:m + 1],
                        scalar2=0.0,
                        op0=ALU.add,
                        op1=ALU.max,
                    )

            def u_phase(b):
                # U = E @ v_ext (col N accumulates sign(alpha)*Z), then
                # out = U * (1/Z) + x fused on the DVE
                o_sb = outp.tile([128, CCH, N], F32, tag="o")
                for m in range(CCH):
                    u_ps = psbank.tile([128, N + 2], F32, tag="psbank")
                    for d in range(CCH):
                        nc.tensor.matmul(
                            u_ps,
                            et[b][d][:, m * 128:(m + 1) * 128],
                            v_sb[b][:, d, :],
                            start=(d == 0),
                            stop=(d == CCH - 1),
                        )
                    rz = smallp.tile([128, 1], F32, tag="rz")
                    nc.vector.reciprocal(out=rz, in_=u_ps[:, N:N + 1])
                    nc.vector.scalar_tensor_tensor(
                        out=o_sb[:, m, :],
                        in0=u_ps[:, 0:N],
                        scalar=rz[:, 0:1],
                        in1=x_sb[b][:, m, :],
                        op0=ALU.mult,
                        op1=ALU.add,
                    )
                    if m % 2 == 1:
                        # stream the result out in 2-chunk pieces so the
                        # store overlaps the remaining compute
                        nc.gpsimd.dma_start(
                            out=out_d[b, :, m - 1:m + 1, :],
                            in_=o_sb[:, m - 1:m + 1, :],
                        )

            # per-sample phase order qk -> [U of prev] -> sim -> v (the
            # empirically best PE stream); loads are split so xt (which
            # gates qk) leads the sync ring and x (residual-only) trails.
            load_xt(0)
            load_xt(1)
            qk_phase(0)
            load_xb(0)
            load_x(0)
            sim_phase(0)
            load_xb(1)
            load_x(1)
            v_phase(0)
            for b in range(1, bs):
                qk_phase(b)
                u_phase(b - 1)
                if b + 1 < bs:
                    load_xt(b + 1)
                    load_xb(b + 1)
                    load_x(b + 1)
                sim_phase(b)
                v_phase(b)
            u_phase(bs - 1)

    nc.compile()
    return nc


def _prep_host(x, Wq, Wk, Wv, bn1_g, bn1_b, bn1_m, bn1_v,
               bn2_g, bn2_b, bn2_m, bn2_v, bn3_g, bn3_b, bn3_m, bn3_v,
               abs_alpha):
    f = np.float32
    s1 = (bn1_g / np.sqrt(bn1_v + EPS)).astype(f)
    t1 = (bn1_b - s1 * bn1_m).astype(f)
    s2 = (bn2_g / np.sqrt(bn2_v + EPS)).astype(f)
    t2 = (bn2_b - s2 * bn2_m).astype(f)
    s3u = (bn3_g / np.sqrt(bn3_v + EPS)).astype(f)
    s3 = s3u * np.float32(abs_alpha)
    t3 = ((bn3_b - s3u * bn3_m) * abs_alpha).astype(f)

    wqk = np.concatenate([Wq * s1[:, None], Wk * s2[:, None]], axis=0).astype(f)
    # lhsT layout [p(=n local), kc, o]
    wqkt = np.ascontiguousarray(
        wqk.T.reshape(KCH, 128, 128).transpose(1, 0, 2)).astype(BFNP)
    tqk = np.concatenate([t1, t2]).reshape(128, 1).astype(f)

    wv2 = (Wv * s3[:, None]).astype(f)
    # lhsT layout [p(=ci local), kc, co]
    wvt = np.ascontiguousarray(
        wv2.T.reshape(CCH, 128, C).transpose(1, 0, 2)).astype(BFNP)
    t3r = np.ascontiguousarray(t3.reshape(CCH, 128).T)

    x = np.asarray(x, dtype=f)
    # [b, p(=c local), kc, n]
    xr = np.ascontiguousarray(x.reshape(B, CCH, 128, N).transpose(0, 2, 1, 3))
    xb = xr.astype(BFNP)
    # [b, p(=n local), kc, c]
    xt = np.ascontiguousarray(
        x.transpose(0, 2, 1).reshape(B, KCH, 128, C).transpose(0, 2, 1, 3)
    ).astype(BFNP)
    return xr, xb, xt, wqkt, tqk, wvt, t3r


def kernel(x, Wq, Wk, Wv,
           bn1_g, bn1_b, bn1_m, bn1_v,
           bn2_g, bn2_b, bn2_m, bn2_v,
           bn3_g, bn3_b, bn3_m, bn3_v,
           alpha):
    global LAST_RESULTS
    args = [np.asarray(a, dtype=np.float32) for a in (
        x, Wq, Wk, Wv, bn1_g, bn1_b, bn1_m, bn1_v,
        bn2_g, bn2_b, bn2_m, bn2_v, bn3_g, bn3_b, bn3_m, bn3_v)]
    alpha_val = float(np.asarray(alpha).reshape(-1)[0])
    if alpha_val == 0.0:
        return np.asarray(x, dtype=np.float32).copy()

    xr, xb, xt, wqkt, tqk, wvt, t3r = _prep_host(*args, abs(alpha_val))
    # |alpha| is folded into v; the ones column carries sign(alpha) so that
    # U * (1/Z) reproduces alpha * (aff @ v)
    vcol = np.full((128, CCH, 2), np.sign(alpha_val), dtype=BFNP)

    if "nc" not in _NC_CACHE:
        _NC_CACHE["nc"] = _build()
    nc = _NC_CACHE["nc"]

    in_maps = []
    for cid in range(NCORES):
        sl = slice(cid * BS, (cid + 1) * BS)
        in_maps.append({
            "x_in": np.ascontiguousarray(xr[sl]),
            "xb_in": np.ascontiguousarray(xb[sl]),
            "xt_in": np.ascontiguousarray(xt[sl]),
            "wqkt": wqkt,
            "tqk": tqk,
            "wvt": wvt,
            "t3": t3r,
            "vcol": vcol,
        })

    trace = bool(int(os.environ.get("KERNEL_TRACE", "0")))
    tmpdir = os.environ.get("KERNEL_TRACE_DIR") or None
    res = run_bass_kernel_spmd(
        nc, in_maps, core_ids=list(range(NCORES)), trace=trace, tmpdir=tmpdir
    )
    LAST_RESULTS = res

    y = np.concatenate([res.results[cid]["y_out"] for cid in range(NCORES)], axis=0)
    y = y.transpose(0, 2, 1, 3).reshape(B, C, N)
    return np.ascontiguousarray(y.astype(np.float32))

